# revision 13
# baseline (speedup 1.0000x reference)
"""Trainium2 Bass kernel for nn_EnergyModel (gnn_message_passing), 8-core SPMD.

Model reduction (validated vs reference, rel-err ~1e-4):
  - Only the scalar (l=0) spherical channel affects the output, so the net is:
      rbf = bernstein(u) * cutoff(r)                    (per edge)
      y1  = segsum_dst((rbf @ (Wy0[0]+Wx0[0])) * x0[src]);  x1 = mlp0(x0, y1)
      y0  = segsum_dst((rbf @ Wr_last) * x1[src]);          xs1 = mlp1(x1, y0)
      e   = readout(xs1) + atomic_bias;  e_graph = segsum(e, batch)
  - Edges with r >= CUTOFF contribute exactly 0 (smooth cutoff) and are
    dropped on the host (neighbor-list style preprocessing).

Distribution (from the sharding hint): nodes partitioned uniformly over the
8 cores by id; each core owns the edges into its nodes (sorted by dst).
The segment-sum is a local degree-grouped strided DVE reduction; the x1
exchange for cut edges is an AllGather of a compact fp16 node table; the
per-graph reduction is done on host from per-core atom energies (tiny).

Device pipeline per core (one SPMD program, data-independent structure via a
cross-core degree envelope):
  feat rows (1,1,L1h,L1l,L2h,L2l,ca,par) -> K=8 fp16 matmul -> exp ->
  rbf2 [64,E] = (rbf*(1-par); rbf*par) -> K=64 matmul -> w1 | w2-pair ->
  msg1 = w1*x0e -> degree-grouped reduce -> MLP0 -> x1 (+ transposed fp16
  table chunks) -> AllGather -> pair-row dma_gather (256B rows, int16 pair
  idx; parity selected by the w2 pair masking) -> msgf = w2*g ->
  reduce -> MLP1 -> readout -> e_atom.

Self-contained: shapes hardcoded from the problem spec; the program is built
at runtime from the actual graph (degree envelope), same on all 8 cores.
"""
import sys
sys.path.insert(0, '/opt/trn_rl_repo')
import numpy as np
from math import lgamma
from contextlib import ExitStack

N_NODES, N_EDGES, N_GRAPHS, F, NB, ZMAX = 50000, 400000, 512, 32, 32, 118
CUTOFF = 5.0
NCORE = 8
NPR = N_NODES // NCORE     # real nodes per core
BIG = np.float32(60.0)     # parity mask shift: exp(-60) underflows fp16 -> 0
CAPAD = np.float32(-30000.0)  # pad-edge ca: exp(arg) == 0 exactly
ET = 512                   # edge matmul tile
GT = 4096                  # gather tile


# ----------------------------------------------------------------- host build
def _build(inp):
    dst = np.asarray(inp['dst_idx']); src = np.asarray(inp['src_idx'])
    pos = np.asarray(inp['positions'], np.float32)
    z = np.asarray(inp['atomic_numbers'])
    embed = np.asarray(inp['embed'], np.float32)
    abias = np.asarray(inp['atomic_bias'], np.float32)

    disp = pos[src] - pos[dst]
    r2 = np.sum(disp * disp, axis=-1) + np.float32(1e-12)
    r = np.sqrt(r2)
    keep = r < CUTOFF
    dst, src, r, r2 = dst[keep], src[keep], r[keep], r2[keep]
    owner = dst // NPR

    deg = np.zeros((NCORE, NPR), np.int64)
    for c in range(NCORE):
        m = owner == c
        deg[c] = np.bincount(dst[m] - c * NPR, minlength=NPR)
    dmax = int(deg.max())
    cnt = np.zeros((NCORE, dmax + 1), np.int64)
    for c in range(NCORE):
        cnt[c] = np.bincount(deg[c], minlength=dmax + 1)
    n_env = cnt.max(axis=0)
    S_env = int(n_env.sum())
    NPC = -(-S_env // 512) * 512   # 512-aligned: node tiles + chunk groups
    slot0 = np.zeros(dmax + 1, np.int64)
    e0 = np.zeros(dmax + 1, np.int64)
    acc_s = acc_e = 0
    for d in range(dmax + 1):
        slot0[d] = acc_s; e0[d] = acc_e
        acc_s += int(n_env[d]); acc_e += int(n_env[d]) * d
    E_env = acc_e
    EPAD = -(-E_env // GT) * GT

    # slot assignment for every node (degree-sorted into envelope groups)
    slot_of = np.empty(N_NODES, np.int64)
    for c in range(NCORE):
        dg = deg[c]
        order = np.argsort(dg, kind='stable')
        sl = np.empty(NPR, np.int64)
        ptr = slot0.copy()
        dgo = dg[order]
        # vectorized per-degree sequential placement
        for d in range(dmax + 1):
            k = int(cnt[c][d])
            if k:
                sel = order[np.searchsorted(dgo, d):np.searchsorted(dgo, d + 1)]
                sl[sel] = np.arange(ptr[d], ptr[d] + k)
        slot_of[c * NPR:(c + 1) * NPR] = sl

    half = NPC // 2
    pair_of = (np.arange(N_NODES) // NPR) * half + (slot_of % half)
    parity_of = slot_of // half

    meta = dict(NPC=NPC, EPAD=EPAD, dmax=dmax, n_env=n_env,
                slot0=slot0, e0=e0, E_env=E_env)

    # log-binomial row for Bernstein basis
    kk = np.arange(NB, dtype=np.float64)
    lb = np.array([lgamma(NB) - lgamma(k + 1) - lgamma(NB - k) for k in kk],
                  np.float32)
    kkf = kk.astype(np.float32)

    # stationary A [8, 64]: feat rows (1, 1, L1h, L1l, L2h, L2l, ca, par)
    A = np.zeros((8, 64), np.float32)
    lbh = lb.astype(np.float16).astype(np.float32)
    for q in range(2):
        s = slice(32 * q, 32 * q + 32)
        A[0, s] = lbh - (BIG if q == 1 else 0.0)
        A[1, s] = lb - lbh
        A[2, s] = kkf; A[3, s] = kkf
        A[4, s] = (NB - 1.0) - kkf; A[5, s] = (NB - 1.0) - kkf
        A[6, s] = 1.0
        A[7, s] = (BIG if q == 1 else -BIG)
    A16 = A.astype(np.float16)

    W0c = np.asarray(inp['Wy0'][0] + inp['Wx0'][0], np.float32)
    Wr = np.asarray(inp['Wr_last'], np.float32)
    Wall = np.zeros((64, 96), np.float32)
    Wall[0:32, 0:32] = W0c; Wall[32:64, 0:32] = W0c
    Wall[0:32, 32:64] = Wr; Wall[32:64, 64:96] = Wr
    Wall16 = Wall.astype(np.float16)

    c0 = float(np.asarray(inp['c0'])[0]); sc0 = c0 / (1 + np.exp(-c0))
    c1 = float(np.asarray(inp['c1'])[0]); sc1 = c1 / (1 + np.exp(-c1))
    W10 = np.asarray(inp['W1_0'], np.float32); b10 = np.asarray(inp['b1_0'])
    W20s = sc0 * np.asarray(inp['W2_0'], np.float32)
    b20s = (sc0 * np.asarray(inp['b2_0'], np.float32))
    W11 = np.asarray(inp['W1_1'], np.float32); b11 = np.asarray(inp['b1_1'])
    W1c = np.concatenate([W11, W11, W11], axis=0)        # [96, 32]
    W21s = sc1 * np.asarray(inp['W2_1'], np.float32)
    b21s = (sc1 * np.asarray(inp['b2_1'], np.float32))
    Wro1 = np.asarray(inp['Wro1'], np.float32); bro1 = np.asarray(inp['bro1'])
    Wro2 = np.asarray(inp['Wro2'], np.float32)
    bro2 = float(np.asarray(inp['bro2'])[0])

    params = dict(
        A=A16, Wall=Wall16,
        W10=W10.astype(np.float16), b10=b10.reshape(F, 1).astype(np.float32),
        W20=W20s.astype(np.float16),
        W1c=W1c.astype(np.float16), b11=b11.reshape(F, 1).astype(np.float32),
        W21=W21s.astype(np.float16),
        Wro1=Wro1.astype(np.float16),
        bro1=bro1.reshape(F, 1).astype(np.float32),
        Wro2=Wro2.reshape(F, 1).astype(np.float16),
    )

    NCH = NPC // 128
    in_maps, posts = [], []
    for c in range(NCORE):
        m = owner == c
        ed = dst[m] - c * NPR; es = src[m]; er = r[m]; er2 = r2[m]
        dg = deg[c]
        sl = slot_of[c * NPR:(c + 1) * NPR]
        eorder = np.argsort(sl[ed], kind='stable')
        ed, es, er, er2 = ed[eorder], es[eorder], er[eorder], er2[eorder]
        sle = sl[ed]; d_of = dg[ed]
        idxs = np.arange(len(ed))
        startmask = np.ones(len(ed), bool); startmask[1:] = sle[1:] != sle[:-1]
        starts = idxs[startmask]
        within = idxs - np.repeat(starts, np.diff(np.append(starts, len(ed))))
        epos = e0[d_of] + (sle - slot0[d_of]) * d_of + within

        feat = np.zeros((8, EPAD), np.float32)
        feat[0] = 1.0; feat[1] = 1.0; feat[6] = CAPAD
        rcp = (np.float32(1.0) / (er + np.float32(1.0))).astype(np.float32)
        u = np.clip(er * rcp, np.float32(1e-7), np.float32(1 - 1e-7))
        L1 = np.log(u); L2 = np.log(rcp)
        a = np.maximum(np.float32(1.0) - er2 / np.float32(CUTOFF * CUTOFF),
                       np.float32(1e-7))
        ca = (np.float32(1.0) - np.float32(1.0) / a).astype(np.float32)
        ca = np.maximum(ca, np.float32(-30000.0))
        L1h = L1.astype(np.float16).astype(np.float32)
        L2h = L2.astype(np.float16).astype(np.float32)
        feat[2, epos] = L1h; feat[3, epos] = L1 - L1h
        feat[4, epos] = L2h; feat[5, epos] = L2 - L2h
        feat[6, epos] = ca
        feat[7, epos] = parity_of[es].astype(np.float32)

        x0e = np.zeros((F, EPAD), np.float16)
        x0e[:, epos] = embed[z[es]].T.astype(np.float16)

        gidx = np.zeros(EPAD, np.int64)
        gidx[epos] = pair_of[es]
        # wrap per GT tile into [128, EPAD//16] (16-partition groups x 8)
        gw = np.zeros((128, EPAD // 16), np.int16)
        for t in range(EPAD // GT):
            blk = gidx[t * GT:(t + 1) * GT].reshape(GT // 16, 16).T  # [16, GT/16]
            for g in range(8):
                gw[16 * g:16 * g + 16, t * (GT // 16):(t + 1) * (GT // 16)] = blk

        gl = np.arange(c * NPR, (c + 1) * NPR)
        x0n = np.zeros((F, NPC), np.float16)
        x0n[:, sl] = embed[z[gl]].T.astype(np.float16)
        x0nTb = np.zeros((NPC, F), np.float32)
        x0nTb[sl] = embed[z[gl]]
        x0nTb += b20s[None, :]          # fold MLP0 output bias into x0^T
        # rearrange to [128, NCH*32] (partition = node%128, cols = (chunk, f))
        x0nTbw = x0nTb.reshape(NCH, 128, F).transpose(1, 0, 2).reshape(128, NCH * F)
        ab = np.zeros((1, NPC), np.float32)
        ab[0, sl] = abias[z[gl]] + bro2
        b21row = np.tile(b21s.reshape(F, 1), (1, 1)).astype(np.float32)

        x0nb = np.zeros((F, NPC), np.float32)
        x0nb[:, sl] = embed[z[gl]].T
        x0nb += b20s[:, None]
        im = dict(params)
        im.update(feat=feat.astype(np.float16), x0e=x0e, gidx=gw,
                  x0n=x0n, x0nb=x0nb, x0nTb=x0nTbw.astype(np.float16),
                  abias=ab, b21=b21row)
        in_maps.append(im)
        posts.append(dict(slot=sl))
    return meta, in_maps, posts


# ------------------------------------------------------------- device program
def _make_nc(meta, reps=1):
    import concourse.bacc as bacc
    import concourse.mybir as mybir
    import concourse.tile as tile
    from concourse.tile_rust import add_dep_helper as adh
    F32 = mybir.dt.float32
    F16 = mybir.dt.float16
    I16 = mybir.dt.int16
    AF = mybir.ActivationFunctionType

    NPC = meta['NPC']; EPAD = meta['EPAD']
    n_env = meta['n_env']; slot0 = meta['slot0']; e0 = meta['e0']
    dmax = meta['dmax']
    NCH = NPC // 128
    TROWS = NCORE * NPC // 2
    NTE = EPAD // ET
    NTG = EPAD // GT
    NTN = -(-NPC // 512)

    nc = bacc.Bacc("TRN2", target_bir_lowering=False, debug=False,
                   enable_asserts=False, num_devices=NCORE)
    t_in = {}
    for name, shape, dt in [
            ('feat', [8, EPAD], F16), ('x0e', [F, EPAD], F16),
            ('gidx', [128, EPAD // 16], I16),
            ('x0n', [F, NPC], F16), ('x0nb', [F, NPC], F32),
            ('x0nTb', [128, NCH * F], F16),
            ('abias', [1, NPC], F32),
            ('A', [8, 64], F16), ('Wall', [64, 96], F16),
            ('W10', [F, F], F16), ('b10', [F, 1], F32),
            ('W20', [F, F], F16),
            ('W1c', [96, F], F16), ('b11', [F, 1], F32),
            ('W21', [F, F], F16), ('b21', [F, 1], F32),
            ('Wro1', [F, F], F16), ('bro1', [F, 1], F32),
            ('Wro2', [F, 1], F16)]:
        t_in[name] = nc.dram_tensor(name, shape, dt, kind="ExternalInput").ap()
    eat_t = nc.dram_tensor("eat", [1, NPC], F32, kind="ExternalOutput").ap()
    tabloc = nc.dram_tensor("tabloc_i", [NPC // 2, 128], F16)
    tabag = nc.dram_tensor("tabag_i", [TROWS, 128], F16, addr_space="Shared")
    w2d = nc.dram_tensor("w2d_i", [64, EPAD], F16)

    with tile.TileContext(nc) as tc, ExitStack() as ctx:
        cpool = ctx.enter_context(tc.tile_pool(name="c", bufs=1))
        epool = ctx.enter_context(tc.tile_pool(name="e", bufs=3))
        gpool = ctx.enter_context(tc.tile_pool(name="g", bufs=2))
        npool = ctx.enter_context(tc.tile_pool(name="n", bufs=2))
        ppool = ctx.enter_context(tc.tile_pool(name="ps", bufs=2, space="PSUM"))

        P = {}
        for name in ['A', 'Wall', 'W10', 'b10', 'W20', 'W1c', 'b11',
                     'W21', 'b21', 'Wro1', 'bro1', 'Wro2', 'x0nTb']:
            t = cpool.tile(list(t_in[name].shape), t_in[name].dtype, tag=name)
            nc.sync.dma_start(t[:], t_in[name][:])
            P[name] = t
        mbuf = cpool.tile([64, EPAD], F16)
        nred = cpool.tile([96, NPC], F32)
        x1 = cpool.tile([F, NPC], F32)

        import os
        PH = int(os.environ.get('KPHASES', '99'))

        def body(_i=None, unroll=None, with_cc=True):
            nc.gpsimd.memset(nred[0:F, :], 0.0)
            # ---- edge phase 1: rbf, projections, msg1; spill w2 to DRAM ----
            for t in range(NTE):
                sl = slice(t * ET, (t + 1) * ET)
                ft = epool.tile([8, ET], F16, tag="feat")
                nc.sync.dma_start(ft[:], t_in['feat'][:, sl])
                argp = ppool.tile([64, ET], F32, tag="arg")
                nc.tensor.matmul(argp[:], P['A'][:], ft[:], start=True, stop=True)
                rbf2 = epool.tile([64, ET], F16, tag="rbf")
                nc.scalar.activation(rbf2[:], argp[:], AF.Exp)
                projp = ppool.tile([96, ET], F32, tag="proj")
                nc.tensor.matmul(projp[:], P['Wall'][:], rbf2[:],
                                 start=True, stop=True)
                x0et = epool.tile([F, ET], F16, tag="x0e")
                nc.sync.dma_start(x0et[:], t_in['x0e'][:, sl])
                wt = epool.tile([96, ET], F16, tag="wt")
                nc.scalar.activation(wt[:], projp[:], AF.Copy)
                nc.vector.tensor_mul(mbuf[0:F, sl], wt[0:F, :], x0et[:])
                nc.sync.dma_start(w2d.ap()[:, sl], wt[F:96, :])
            # ---- reduce 1 into nred[0:32] ----
            if PH < 2:
                nc.sync.dma_start(eat_t[:, 0:512], nred[0:1, 0:512]); return
            for d in range(1, dmax + 1):
                n = int(n_env[d])
                if n == 0:
                    continue
                s0 = int(slot0[d]); ee = int(e0[d])
                nc.vector.tensor_reduce(
                    nred[0:F, s0:s0 + n],
                    mbuf[0:F, ee:ee + n * d].rearrange("p (n d) -> p n d", d=d),
                    axis=mybir.AxisListType.X, op=mybir.AluOpType.add)
            # ---- node MLP0 -> x1; transposed table chunks ----
            if PH < 3:
                nc.sync.dma_start(eat_t[:, 0:512], nred[0:1, 0:512]); return
            wtabs = []
            for j in range(NTN):
                w = min(512, NPC - j * 512)
                sl = slice(j * 512, j * 512 + w)
                x0t = npool.tile([F, 512], F16, tag="x0t")
                nc.sync.dma_start(x0t[:, :w], t_in['x0n'][:, sl])
                t0h = npool.tile([F, 512], F16, tag="t0h")
                nc.vector.tensor_add(t0h[:, :w], nred[0:F, sl], x0t[:, :w])
                p1 = ppool.tile([F, 512], F32, tag="npA")
                nc.tensor.matmul(p1[:, :w], P['W10'][:], t0h[:, :w],
                                 start=True, stop=True)
                ga = npool.tile([F, 512], F16, tag="ga")
                nc.scalar.activation(ga[:, :w], p1[:, :w], AF.Silu,
                                     bias=P['b10'][:])
                p2 = ppool.tile([F, 512], F32, tag="npB")
                nc.tensor.matmul(p2[:, :w], P['W20'][:], ga[:, :w],
                                 start=True, stop=True)
                x0bt = npool.tile([F, 512], F32, tag="x0bt")
                nc.sync.dma_start(x0bt[:, :w], t_in['x0nb'][:, sl])
                nc.vector.tensor_add(x1[:, sl], p2[:, :w], x0bt[:, :w])
                # table chunks for this tile (512 = 4 x 128)
                pT = ppool.tile([128, 128], F32, tag="npA")
                for q in range(4):
                    nc.tensor.matmul(pT[:, q * F:(q + 1) * F],
                                     ga[:, q * 128:(q + 1) * 128],
                                     P['W20'][:], start=True, stop=True)
                pTh = npool.tile([128, 128], F16, tag="pTh")
                nc.vector.tensor_copy(pTh[:], pT[:])
                x1T = npool.tile([128, 128], F16, tag="x1T")
                nc.vector.tensor_add(
                    x1T[:], pTh[:], P['x0nTb'][:, j * 128:(j + 1) * 128])
                for q in range(4):
                    s0c = j * 512 + q * 128
                    half = NPC // 2
                    row = s0c % half
                    cb = (s0c // half) * F
                    wi = nc.sync.dma_start(
                        tabloc.ap()[row:row + 128, cb:cb + F],
                        x1T[:, q * F:(q + 1) * F])
                    wtabs.append(wi)
            # ---- AllGather (pair rows are gather-ready) ----
            if PH < 4:
                nc.sync.dma_start(eat_t[:, 0:512], x1[0:1, 0:512]); return
            if with_cc:
                cc = nc.gpsimd.collective_compute(
                    "AllGather", mybir.AluOpType.bypass,
                    replica_groups=[list(range(NCORE))],
                    ins=[tabloc.ap()], outs=[tabag.ap()])
                for wi in wtabs:
                    adh(cc.ins, wi.ins, reason="allgather after table writes")
            else:
                cc = wtabs[-1]
            # ---- gather + msgf ----
            if PH < 5:
                nc.sync.dma_start(eat_t[:, 0:512], x1[0:1, 0:512]); return
            rp = cc
            gsrc = tabag.ap()
            for g in range(NTG):
                sl = slice(g * GT, (g + 1) * GT)
                ixt = gpool.tile([128, GT // 16], I16, tag="ix")
                nc.sync.dma_start(
                    ixt[:], t_in['gidx'][:, g * (GT // 16):(g + 1) * (GT // 16)])
                gout = gpool.tile([128, 1, GT], F16, tag="gout")
                gi = nc.gpsimd.dma_gather(gout[:], gsrc, ixt[:],
                                          GT, GT, elem_size=128, transpose=True,
                                          single_packet=False)
                adh(gi.ins, rp.ins, reason="gather after repack")
                w2t = gpool.tile([64, GT], F16, tag="w2t")
                nc.sync.dma_start(w2t[:], w2d.ap()[:, sl])
                nc.vector.tensor_mul(mbuf[:, sl], w2t[:], gout[0:64, 0, :])
            # ---- reduce 2 into nred[0:64] ----
            if PH < 6:
                nc.sync.dma_start(eat_t[:, 0:512], x1[0:1, 0:512]); return
            nc.gpsimd.memset(nred[0:64, :], 0.0)
            for d in range(1, dmax + 1):
                n = int(n_env[d])
                if n == 0:
                    continue
                s0 = int(slot0[d]); ee = int(e0[d])
                nc.vector.tensor_reduce(
                    nred[0:64, s0:s0 + n],
                    mbuf[:, ee:ee + n * d].rearrange("p (n d) -> p n d", d=d),
                    axis=mybir.AxisListType.X, op=mybir.AluOpType.add)
            nc.scalar.activation(nred[64:96, :], x1[:], AF.Copy)
            # ---- node MLP1 -> xs1; readout ----
            for j in range(NTN):
                w = min(512, NPC - j * 512)
                sl = slice(j * 512, j * 512 + w)
                asmt = npool.tile([96, 512], F16, tag="asmt")
                nc.scalar.activation(asmt[:, :w], nred[:, sl], AF.Copy)
                p1 = ppool.tile([F, 512], F32, tag="npA")
                nc.tensor.matmul(p1[:, :w], P['W1c'][:], asmt[:, :w],
                                 start=True, stop=True)
                ga = npool.tile([F, 512], F16, tag="ga2")
                nc.scalar.activation(ga[:, :w], p1[:, :w], AF.Silu,
                                     bias=P['b11'][:])
                p2 = ppool.tile([F, 512], F32, tag="npB")
                nc.tensor.matmul(p2[:, :w], P['W21'][:], ga[:, :w],
                                 start=True, stop=True)
                tmp = npool.tile([F, 512], F32, tag="ntmp")
                nc.vector.tensor_scalar_add(tmp[:, :w], p2[:, :w], P['b21'][:])
                xs1h = npool.tile([F, 512], F16, tag="xs1h")
                nc.vector.tensor_add(xs1h[:, :w], tmp[:, :w], x1[:, sl])
                p3 = ppool.tile([F, 512], F32, tag="npA")
                nc.tensor.matmul(p3[:, :w], P['Wro1'][:], xs1h[:, :w],
                                 start=True, stop=True)
                h = npool.tile([F, 512], F16, tag="h")
                nc.scalar.activation(h[:, :w], p3[:, :w], AF.Silu,
                                     bias=P['bro1'][:])
                p4 = ppool.tile([1, 512], F32, tag="npB")
                nc.tensor.matmul(p4[:, :w], P['Wro2'][:], h[:, :w],
                                 start=True, stop=True)
                abt = npool.tile([1, 512], F32, tag="abt")
                nc.sync.dma_start(abt[:, :w], t_in['abias'][:, sl])
                et = npool.tile([1, 512], F32, tag="et")
                nc.vector.tensor_add(et[:, :w], p4[:, :w], abt[:, :w])
                nc.sync.dma_start(eat_t[:, sl], et[:, :w])

        if reps == 1:
            body()
        else:
            # collectives are unsupported inside For_i on this runtime:
            # run the full body once (fills tabag), then loop without the cc
            body()
            with tc.For_i(0, reps, 1) as i:
                body(i, with_cc=False)
    nc.compile()
    return nc


def _make_cc_bench(meta, n):
    """NEFF with n sequential AllGathers of the table (timing only)."""
    import concourse.bacc as bacc
    import concourse.mybir as mybir
    import concourse.tile as tile
    from contextlib import ExitStack
    F16 = mybir.dt.float16
    F32 = mybir.dt.float32
    NPC = meta['NPC']
    TROWS = NCORE * NPC // 2
    nc = bacc.Bacc("TRN2", target_bir_lowering=False, debug=False,
                   enable_asserts=False, num_devices=NCORE)
    src_t = nc.dram_tensor("src", [NPC // 2, 128], F16, kind="ExternalInput").ap()
    o_t = nc.dram_tensor("o", [1, 128], F32, kind="ExternalOutput").ap()
    tabloc = nc.dram_tensor("tabloc_i", [NPC // 2, 128], F16)
    tabag = nc.dram_tensor("tabag_i", [TROWS, 128], F16, addr_space="Shared")
    with tile.TileContext(nc) as tc, ExitStack() as ctx:
        pool = ctx.enter_context(tc.tile_pool(name="p", bufs=2))
        nc.sync.dma_start(tabloc.ap(), src_t[:])
        for _ in range(n):
            nc.gpsimd.collective_compute(
                "AllGather", mybir.AluOpType.bypass,
                replica_groups=[list(range(NCORE))],
                ins=[tabloc.ap()], outs=[tabag.ap()])
        t = pool.tile([1, 128], F32)
        nc.gpsimd.memset(t[:], 1.0)
        nc.sync.dma_start(o_t[:], t[:])
    nc.compile()
    return nc


_CACHE = {}


def _get_nc(meta, reps=1):
    key = (meta['NPC'], meta['EPAD'], meta['dmax'],
           tuple(int(x) for x in meta['n_env']), reps)
    if key not in _CACHE:
        _CACHE[key] = _make_nc(meta, reps)
    return _CACHE[key]


def kernel(**inputs):
    from concourse.bass_utils import run_bass_kernel_spmd
    inp = {k: np.asarray(v) for k, v in inputs.items()}
    meta, in_maps, posts = _build(inp)
    nc = _get_nc(meta, reps=1)
    res = run_bass_kernel_spmd(nc, in_maps, list(range(NCORE)), trace=False)
    batch_seg = np.asarray(inp['batch_segments'])
    e_graph = np.zeros(N_GRAPHS, np.float64)
    for c in range(NCORE):
        eat = np.asarray(res.results[c]['eat'][0], np.float64)
        gl = np.arange(c * NPR, (c + 1) * NPR)
        e_real = eat[posts[c]['slot']]
        e_graph += np.bincount(batch_seg[gl], weights=e_real,
                               minlength=N_GRAPHS)
    out = np.where(np.asarray(inp['graph_mask']), e_graph, 0.0).astype(np.float32)
    return out
